# revision 1
# baseline (speedup 1.0000x reference)
"""Causal flash attention for Trainium2, sharded 2 heads/core over 8 cores.

Math per head: out = softmax_causal(Q K^T / sqrt(D)) @ V,  Q/K/V [S=2048, D=64] fp32.

Device layout (per core, heads h0=2c, h1=2c+1):
  qT   [128, 2048]  rows 64h+d = Q[h]^T        (D on partitions, both heads stacked)
  kT   [128, 2048]  same for K
  vaug [2, 128, 1040] vaug[h, p, 65*kc+d] = V[h, 128*kc+p, d], d=64 column is ones
  outT [2, 64, 2048]  out[h]^T (normalized)

Scores are computed transposed (S^T[k, q] = K_chunk @ Q^T) so no transposes are
needed anywhere: softmax denominator comes out of the PV matmul via the ones
column of vaug (psum row 64), and the final division is done by broadcasting
1/denom across partitions with a K=1 matmul against a ones vector.
"""

import os
import sys

import ml_dtypes
import numpy as np

sys.path.insert(0, "/opt/trn_rl_repo")

import concourse.bass as bass
import concourse.bacc as bacc
import concourse.mybir as mybir
import concourse.tile as tile
from concourse.bass_utils import run_bass_kernel_spmd

B, H, S, D = 1, 16, 2048, 64
N_CORES = 8
HEADS_PER_CORE = H // N_CORES  # 2
N_CHUNKS = S // 128  # 16 key chunks per head
N_SPANS = S // 512  # 4 query spans per head
F32 = mybir.dt.float32
R32 = mybir.dt.float32r  # one-pass fp32 matmul mode (TF32-like); 2x PE throughput
BF16 = mybir.dt.bfloat16
F16 = mybir.dt.float16
NEG = -1.0e30


def _r(ap):
    return ap.bitcast(R32)

_NC = None
_LAST_RESULTS = None


def _build_bass():
    nc = bacc.Bacc("TRN2", target_bir_lowering=False)
    qT = nc.declare_dram_parameter("qT", [128, S], F16, isOutput=False)
    kT = nc.declare_dram_parameter("kT", [128, S], F16, isOutput=False)
    vaug = nc.declare_dram_parameter("vaug", [2, 128, 65 * N_CHUNKS], BF16, isOutput=False)
    outT = nc.declare_dram_parameter("outT", [2, 64, S], F32, isOutput=True)

    with tile.TileContext(nc) as tc:
        with (
            tc.tile_pool(name="const", bufs=1) as const,
            tc.tile_pool(name="inbuf", bufs=1) as inbuf,
            tc.tile_pool(name="pbuf", bufs=6) as pbuf,
            tc.tile_pool(name="nbuf", bufs=4) as nbuf,
            tc.tile_pool(name="ps_s", bufs=3, space="PSUM") as ps_s,
            tc.tile_pool(name="ps_o", bufs=1, space="PSUM") as ps_o,
        ):
            # Input loads, chunked by 512 query/key columns so compute can
            # start before all DMAs land.
            qsb = [None] * N_SPANS
            ksb = [None] * N_SPANS
            vsb = [[None] * N_SPANS, [None] * N_SPANS]
            # First span processed is s=3 (needs q3 + all of k/v in kc order).
            qsb[3] = inbuf.tile([128, 512], F16, tag="q3", name="qt3")
            nc.sync.dma_start(out=qsb[3], in_=qT[:, 1536:2048])
            for j in range(N_SPANS):
                kt = inbuf.tile([128, 512], F16, tag=f"k{j}", name=f"kt{j}")
                nc.sync.dma_start(out=kt, in_=kT[:, 512 * j : 512 * (j + 1)])
                ksb[j] = kt
                for h in range(2):
                    vt = inbuf.tile([128, 4 * 65], BF16, tag=f"v{h}{j}", name=f"vt{h}{j}")
                    nc.sync.dma_start(
                        out=vt, in_=vaug[h, :, 260 * j : 260 * (j + 1)]
                    )
                    vsb[h][j] = vt
            for j in (2, 1, 0):
                qt = inbuf.tile([128, 512], F16, tag=f"q{j}", name=f"qt{j}")
                nc.sync.dma_start(out=qt, in_=qT[:, 512 * j : 512 * (j + 1)])
                qsb[j] = qt

            def k_slice(h, kc):
                # kT chunk [64, 128] for head h: lhsT of the scores matmul.
                return ksb[kc // 4][64 * h : 64 * h + 64, 128 * (kc % 4) : 128 * (kc % 4) + 128]

            def q_slice(h, qs, qe):
                j = qs // 512
                base = 512 * j
                return qsb[j][64 * h : 64 * h + 64, qs - base : qe - base]

            def v_slice(h, kc):
                return vsb[h][kc // 4][:, 65 * (kc % 4) : 65 * (kc % 4) + 65]

            def emit_chunks(s, heads, po):
                qs, qe = 512 * s, 512 * (s + 1)
                nkc = 4 * s + 4
                for kc in range(nkc):
                    qb = max(qs, 128 * kc)
                    w = qe - qb
                    # One psum tile holds both heads' score blocks: h0 at
                    # cols 0:512 (bank A), h1 at 512:1024 (bank B). The two
                    # matmuls hit disjoint PE row groups (0-63 / 64-127) and
                    # run concurrently, draining into different banks. One
                    # exp covers both blocks; for narrow (diag) chunks it
                    # also reads the stale gap [w:512], which is harmless
                    # (finite stale scores, never consumed).
                    pg = ps_s.tile([128, 1024], F32, tag="pss", name=f"pg_{s}_{kc}_{heads[0]}")
                    pe2 = pbuf.tile([128, 1024], BF16, tag="pe", name=f"pe_{s}_{kc}_{heads[0]}")
                    off1 = 512
                    for i, h in enumerate(heads):
                        nc.tensor.matmul(
                            pg[:, off1 * i : off1 * i + w],
                            k_slice(h, kc),
                            q_slice(h, qb, qe),
                            start=True,
                            stop=True,
                        )
                    fd = off1 * (len(heads) - 1) + w
                    nc.scalar.activation(
                        out=pe2[:, :fd],
                        in_=pg[:, :fd],
                        func=mybir.ActivationFunctionType.Exp,
                        scale=0.125,
                    )
                    if kc >= 4 * s:
                        # diagonal chunk: zero the strict upper triangle
                        # (q < k) of the first 128 cols, after exp, on the
                        # otherwise-idle gpsimd engine.
                        for i, h in enumerate(heads):
                            nc.gpsimd.affine_select(
                                out=pe2[:, off1 * i : off1 * i + 128],
                                in_=pe2[:, off1 * i : off1 * i + 128],
                                compare_op=mybir.AluOpType.is_ge,
                                fill=0.0,
                                base=0,
                                pattern=[[1, 128]],
                                channel_multiplier=-1,
                            )
                    for i, h in enumerate(heads):
                        nc.tensor.matmul(
                            po[h][:, qb - qs : qb - qs + w],
                            v_slice(h, kc),
                            pe2[:, off1 * i : off1 * i + w],
                            start=(kc == 0),
                            stop=(kc == nkc - 1),
                        )

            def emit_tail(s, h, po):
                qs, qe = 512 * s, 512 * (s + 1)
                if True:
                    # Copy accumulator (+ denom row 64) to SBUF immediately so
                    # the psum bank frees for the next span; normalize from the
                    # SBUF copy.
                    ou = nbuf.tile([65, 512], F32, tag="ou", name=f"ou{h}_{s}")
                    nc.vector.tensor_copy(out=ou, in_=po[h][:, :])
                    # 1/denom: DVE reciprocal is ~8 cyc per free-dim element
                    # (lanes parallel, FD serial), so reshape the 512 values
                    # onto 128 partitions via a small SBUF->SBUF DMA round
                    # trip and run reciprocal at FD=4.
                    d4 = nbuf.tile([128, 4], F32, tag="d4")
                    nc.sync.dma_start(out=d4, in_=ou[64:65, :])
                    r4 = nbuf.tile([128, 4], F32, tag="r4")
                    nc.vector.reciprocal(out=r4, in_=d4)
                    r_sb = nbuf.tile([1, 512], F32, tag="r")
                    nc.sync.dma_start(out=r_sb, in_=r4)
                    rb_sb = nbuf.tile([64, 512], F32, tag="rb")
                    nc.gpsimd.partition_broadcast(rb_sb[:, :], r_sb[0:1, :])
                    o_sb = nbuf.tile([64, 512], F32, tag="o")
                    nc.vector.tensor_mul(out=o_sb, in0=ou[0:64, :], in1=rb_sb)
                    nc.sync.dma_start(out=outT[h, :, qs:qe], in_=o_sb)

            for s in (3, 2, 1, 0):
                po = [ps_o.tile([65, 512], F32, tag=f"po{hh}", name=f"po{hh}_{s}") for hh in range(2)]
                emit_chunks(s, (0, 1), po)
                for h in range(2):
                    emit_tail(s, h, po)

    nc.compile()
    return nc


def _get_nc():
    global _NC
    if _NC is None:
        _NC = _build_bass()
    return _NC


def kernel(q, k, v):
    global _LAST_RESULTS
    q = np.asarray(q, dtype=np.float32)
    k = np.asarray(k, dtype=np.float32)
    v = np.asarray(v, dtype=np.float32)
    assert q.shape == (B, H, S, D)

    in_maps = []
    for c in range(N_CORES):
        h0 = HEADS_PER_CORE * c
        qT = np.ascontiguousarray(
            q[0, h0 : h0 + 2].transpose(0, 2, 1).reshape(128, S)
        ).astype(np.float16)
        kT = np.ascontiguousarray(
            k[0, h0 : h0 + 2].transpose(0, 2, 1).reshape(128, S)
        ).astype(np.float16)
        va = np.ones((2, 128, N_CHUNKS, 65), dtype=np.float32)
        va[..., :64] = (
            v[0, h0 : h0 + 2].reshape(2, N_CHUNKS, 128, 64).transpose(0, 2, 1, 3)
        )
        va16 = va.reshape(2, 128, 65 * N_CHUNKS).astype(ml_dtypes.bfloat16)
        in_maps.append({"qT": qT, "kT": kT, "vaug": va16})

    nc = _get_nc()
    res = run_bass_kernel_spmd(nc, in_maps, core_ids=list(range(N_CORES)))
    _LAST_RESULTS = res

    out = np.empty((B, H, S, D), dtype=np.float32)
    for c in range(N_CORES):
        ot = res.results[c]["outT"]  # [2, 64, 2048]
        out[0, 2 * c] = ot[0].T
        out[0, 2 * c + 1] = ot[1].T
    return out



# revision 6
# speedup vs baseline: 1.0048x; 1.0048x over previous
"""Causal flash attention for Trainium2, sharded 2 heads/core over 8 cores.

Math per head: out = softmax_causal(Q K^T / sqrt(D)) @ V,  Q/K/V [S=2048, D=64] fp32.

Device layout (per core, heads h0=2c, h1=2c+1):
  qT   [128, 2048]  rows 64h+d = Q[h]^T        (D on partitions, both heads stacked)
  kT   [128, 2048]  same for K
  vaug [2, 128, 1040] vaug[h, p, 65*kc+d] = V[h, 128*kc+p, d], d=64 column is ones
  outT [2, 64, 2048]  out[h]^T (normalized)

Scores are computed transposed (S^T[k, q] = K_chunk @ Q^T) so no transposes are
needed anywhere: softmax denominator comes out of the PV matmul via the ones
column of vaug (psum row 64), and the final division broadcasts 1/denom across
partitions with gpsimd.partition_broadcast.

The kernel is paced by the Scalar (ACT) engine's exp over every causal score.
Structure is chosen to keep ACT at its column floor:
  - scores land in psum [128,1024] tiles (h0 in bank A, h1 in bank B), one exp
    instruction per tile covering both heads.
  - diagonal chunks (width w < 512) are END-aligned against the bank boundary
    (h0 at [512-w:512], h1 at [512:512+w]) so the exp covers 2w contiguous
    columns with zero gap waste; the two smallest diagonal chunks of each span
    share one tile, giving 36 ACTIVATEs and exactly S^2/2-worth of columns.
  - causal triangle masks run on the DVE (multiply by a precomputed mask) so
    the Scalar queue carries nothing but the exps.
"""

import os
import sys

import ml_dtypes
import numpy as np

sys.path.insert(0, "/opt/trn_rl_repo")

import concourse.bass as bass
import concourse.bacc as bacc
import concourse.mybir as mybir
import concourse.tile as tile
from concourse.bass_utils import run_bass_kernel_spmd

B, H, S, D = 1, 16, 2048, 64
N_CORES = 8
HEADS_PER_CORE = H // N_CORES  # 2
N_CHUNKS = S // 128  # 16 key chunks per head
N_SPANS = S // 512  # 4 query spans per head
F32 = mybir.dt.float32
BF16 = mybir.dt.bfloat16
F16 = mybir.dt.float16

_NC = None
_LAST_RESULTS = None


def _build_bass():
    nc = bacc.Bacc("TRN2", target_bir_lowering=False)
    qT = nc.declare_dram_parameter("qT", [128, S], F16, isOutput=False)
    kT = nc.declare_dram_parameter("kT", [128, S], F16, isOutput=False)
    vaug = nc.declare_dram_parameter("vaug", [2, 128, 65 * N_CHUNKS], BF16, isOutput=False)
    outT = nc.declare_dram_parameter("outT", [2, 64, S], F32, isOutput=True)

    with tile.TileContext(nc) as tc:
        with (
            tc.tile_pool(name="const", bufs=1) as const,
            tc.tile_pool(name="inbuf", bufs=1) as inbuf,
            tc.tile_pool(name="pbuf", bufs=6) as pbuf,
            tc.tile_pool(name="nbuf", bufs=2) as nbuf,
            tc.tile_pool(name="ps_s", bufs=2, space="PSUM") as ps_s,
            tc.tile_pool(name="ps_o", bufs=2, space="PSUM") as ps_o,
        ):
            # Constants: a zeros tile for PE warmup matmuls and the causal
            # triangle mask (keep k <= q within a 128x128 diagonal block).
            zwarm = const.tile([64, 512], BF16, tag="zw", name="zwarm")
            nc.gpsimd.memset(zwarm, 0.0)

            # Input loads: few big DMAs. Only SP/gpsimd queues can issue DMAs
            # (Activation is the pacing engine - keep it clean). k0 + q3 gate
            # the first matmul; the rest stream in while span 3 computes.
            k0 = inbuf.tile([128, 512], F16, tag="k0", name="k0")
            nc.sync.dma_start(out=k0, in_=kT[:, 0:512])
            q3 = inbuf.tile([128, 512], F16, tag="q3", name="q3")
            nc.sync.dma_start(out=q3, in_=qT[:, 1536:2048])
            v0 = inbuf.tile([128, 65 * N_CHUNKS], BF16, tag="v0", name="v0")
            nc.gpsimd.dma_start(out=v0, in_=vaug[0])
            v1 = inbuf.tile([128, 65 * N_CHUNKS], BF16, tag="v1", name="v1")
            nc.gpsimd.dma_start(out=v1, in_=vaug[1])
            krest = inbuf.tile([128, 1536], F16, tag="kr", name="krest")
            nc.sync.dma_start(out=krest, in_=kT[:, 512:2048])
            q012 = inbuf.tile([128, 1536], F16, tag="q012", name="q012")
            nc.sync.dma_start(out=q012, in_=qT[:, 0:1536])
            vsb = [v0, v1]

            mtri = const.tile([128, 128], BF16, tag="mtri", name="mtri")
            nc.gpsimd.memset(mtri, 1.0)
            nc.gpsimd.affine_select(
                out=mtri,
                in_=mtri,
                compare_op=mybir.AluOpType.is_ge,
                fill=0.0,
                base=0,
                pattern=[[1, 128]],
                channel_multiplier=-1,
            )

            def k_slice(h, kc):
                # kT chunk [64, 128] for head h: lhsT of the scores matmul.
                if kc < 4:
                    return k0[64 * h : 64 * h + 64, 128 * kc : 128 * kc + 128]
                c = 128 * kc - 512
                return krest[64 * h : 64 * h + 64, c : c + 128]

            def q_slice(h, qs, qe):
                if qs >= 1536:
                    return q3[64 * h : 64 * h + 64, qs - 1536 : qe - 1536]
                assert qe <= 1536
                return q012[64 * h : 64 * h + 64, qs:qe]

            def v_slice(h, kc):
                return vsb[h][:, 65 * kc : 65 * kc + 65]

            # PE warmup: burn the HAM cold window during the input-DMA wait so
            # real matmuls run at 2.4 GHz. Results are overwritten (start=True)
            # by the real score matmuls on the same tile addresses.
            for i in range(3):
                pw = ps_s.tile([128, 1024], F32, tag="pss", name=f"warm{i}")
                nc.tensor.matmul(pw[:, 0:512], zwarm[:, 0:128], zwarm, start=True, stop=True)
                nc.tensor.matmul(pw[:, 512:1024], zwarm[:, 0:128], zwarm, start=True, stop=True)

            def emit_span(s, po):
                qs, qe = 512 * s, 512 * (s + 1)
                # Tiles: non-diag chunks kc<4s get one [128,1024] tile each
                # (h0 block [0:512] bank A, h1 [512:1024] bank B). Diagonal
                # chunks are end-aligned: h0 at [512-w:512], h1 at [512:512+w],
                # so the exp covers 2w contiguous columns with no gap waste.
                # The two smallest diagonal chunks (w=256,128) share one tile.
                tiles = [[kc] for kc in range(4 * s)]
                tiles.append([4 * s])      # diag w=512 (full block, needs mask)
                tiles.append([4 * s + 1])  # diag w=384
                tiles.append([4 * s + 2, 4 * s + 3])  # diag w=256,128 merged
                for tix, kcs in enumerate(tiles):
                    diag = kcs[0] >= 4 * s
                    pg = ps_s.tile([128, 1024], F32, tag="pss", name=f"pg_{s}_{kcs[0]}")
                    pe2 = pbuf.tile([128, 1024], BF16, tag="pe", name=f"pe_{s}_{kcs[0]}")
                    ws = [qe - max(qs, 128 * kc) for kc in kcs]
                    tot = sum(ws)
                    blocks = []  # (h, kc, off, w)
                    o = 512 - tot
                    for kc, w in reversed(list(zip(kcs, ws))):
                        blocks.append((0, kc, o, w))
                        o += w
                    o = 512
                    for kc, w in zip(kcs, ws):
                        blocks.append((1, kc, o, w))
                        o += w
                    lo, hi = 512 - tot, 512 + tot
                    # Score matmuls: per kc the (h0, h1) pair runs concurrently
                    # on PE row groups 0-63 / 64-127, draining into banks A/B.
                    order = sorted(blocks, key=lambda b: (b[1], b[0]))
                    for h, kc, off, w in order:
                        qb = qe - w
                        nc.tensor.matmul(
                            pg[:, off : off + w],
                            k_slice(h, kc),
                            q_slice(h, qb, qe),
                            start=True,
                            stop=True,
                        )
                    # One exp for the whole tile (both heads, all its chunks).
                    nc.scalar.activation(
                        out=pe2[:, lo:hi],
                        in_=pg[:, lo:hi],
                        func=mybir.ActivationFunctionType.Exp,
                        scale=0.125,
                    )
                    # Causal masks for diagonal chunks: zero the strict upper
                    # triangle of each block's first 128 query columns (DVE).
                    if diag:
                        for h, kc, off, w in order:
                            nc.vector.tensor_mul(
                                out=pe2[:, off : off + 128],
                                in0=pe2[:, off : off + 128],
                                in1=mtri,
                            )
                    # PV accumulation.
                    nkc = 4 * s + 4
                    for h, kc, off, w in order:
                        qb = qe - w
                        nc.tensor.matmul(
                            po[h][:, qb - qs : qb - qs + w],
                            v_slice(h, kc),
                            pe2[:, off : off + w],
                            start=(kc == 0),
                            stop=(kc == nkc - 1),
                        )

            def emit_tail(s, h, po):
                qs, qe = 512 * s, 512 * (s + 1)
                # 1/denom: copy the psum denominator row to SBUF (custom DVE
                # ops read PSUM incorrectly), fast approx reciprocal (~51
                # ULP), broadcast across partitions on the otherwise-idle
                # gpsimd, normalize on DVE, store via sync.
                dn = nbuf.tile([1, 512], F32, tag=f"dn{h}", name=f"dn{h}_{s}")
                nc.vector.tensor_copy(out=dn, in_=po[h][64:65, :])
                r = nbuf.tile([1, 512], F32, tag=f"r{h}", name=f"r{h}_{s}")
                nc.vector.reciprocal_approx_fast(out=r, in_=dn)
                rb = nbuf.tile([64, 512], F32, tag=f"rb{h}", name=f"rb{h}_{s}")
                nc.gpsimd.partition_broadcast(rb[:, :], r[0:1, :])
                o_sb = nbuf.tile([64, 512], F32, tag=f"o{h}", name=f"o{h}_{s}")
                nc.vector.tensor_mul(out=o_sb, in0=po[h][0:64, :], in1=rb)
                nc.sync.dma_start(out=outT[h, :, qs:qe], in_=o_sb)

            for s in (3, 2, 1, 0):
                po = [
                    ps_o.tile([65, 512], F32, tag=f"po{hh}", name=f"po{hh}_{s}")
                    for hh in range(2)
                ]
                emit_span(s, po)
                for h in range(2):
                    emit_tail(s, h, po)

    nc.compile()
    return nc


def _get_nc():
    global _NC
    if _NC is None:
        _NC = _build_bass()
    return _NC


def kernel(q, k, v):
    global _LAST_RESULTS
    q = np.asarray(q, dtype=np.float32)
    k = np.asarray(k, dtype=np.float32)
    v = np.asarray(v, dtype=np.float32)
    assert q.shape == (B, H, S, D)

    in_maps = []
    for c in range(N_CORES):
        h0 = HEADS_PER_CORE * c
        qTh = np.ascontiguousarray(
            q[0, h0 : h0 + 2].transpose(0, 2, 1).reshape(128, S)
        ).astype(np.float16)
        kTh = np.ascontiguousarray(
            k[0, h0 : h0 + 2].transpose(0, 2, 1).reshape(128, S)
        ).astype(np.float16)
        va = np.ones((2, 128, N_CHUNKS, 65), dtype=np.float32)
        va[..., :64] = (
            v[0, h0 : h0 + 2].reshape(2, N_CHUNKS, 128, 64).transpose(0, 2, 1, 3)
        )
        va16 = va.reshape(2, 128, 65 * N_CHUNKS).astype(ml_dtypes.bfloat16)
        in_maps.append({"qT": qTh, "kT": kTh, "vaug": va16})

    nc = _get_nc()
    res = run_bass_kernel_spmd(nc, in_maps, core_ids=list(range(N_CORES)))
    _LAST_RESULTS = res

    out = np.empty((B, H, S, D), dtype=np.float32)
    for c in range(N_CORES):
        ot = res.results[c]["outT"]  # [2, 64, 2048]
        out[0, 2 * c] = ot[0].T
        out[0, 2 * c + 1] = ot[1].T
    return out
